# revision 1
# baseline (speedup 1.0000x reference)
"""ALNN layer kernel for 8 TRN2 NeuronCores (raw Bass, explicit semaphores).

out[b,r,d] = relu( sum_l w_v[r,l,d]*relu(z[b,r,l,d]) + L*b_v[r,d] )
z = wt0*X + wt1*relu(X)*k + wt2*M + wt3*PD + 4*bt
k = exp(-relu(alpha_r)*|T - s_r|)        (uses relu(X*k) == relu(X)*k, k>0)

Sharding: B split 2 ways x R split 4 ways -> 8 cores, each 16 b x 12 r.
Per-core layout: partitions = L(128), free = (b=16, d=64) = 1024.

v2 engine plan (v1 profiling: GpSimd shares an SBUF port with DVE — an
exclusive lock — so streaming on POOL poisoned DVE ops 677ns -> 2600ns):
 - DVE: only the 5 elementwise products (+ wl = w_v*lat), bf16 2x mode
 - PE:  z = bt4 + q + m0 + m2 + m3 via identity-matmul PSUM accumulation
        (bt4 host-expanded to [L, B*D] as the PSUM initializer), plus the
        final L-reduction as one-hot-column matmuls into PSUM row r
 - ACT: dist=abs, k=exp, lat=relu(PSUM z) (ACT is closest to PSUM)
 - POOL: nothing.

Raw bass: this toolchain's walrus allows at most ONE attached sync-wait
per compute instruction, so cross-engine deps use standalone wait_ge
instructions; DMA completion uses per-stream/per-slot semaphores (HW DMA
queues complete out of order, so one shared counting sem is unsound).
"""

import os
import numpy as np
import ml_dtypes

import concourse.bass as bass
import concourse.mybir as mybir
from concourse.bass_utils import run_bass_kernel_spmd

AF = mybir.ActivationFunctionType
OP = mybir.AluOpType
BF16 = mybir.dt.bfloat16
F32 = mybir.dt.float32

B, R, L, D = 32, 48, 128, 64
NB, NR = 2, 4              # b-blocks x r-blocks = 8 cores
BC, RC = B // NB, R // NR  # 16, 12 per core
FD = BC * D                # 1024 free elems

# packed f32 const layout: [Tt 1024 | Abc 12 | NASbc 12 | LBV 64(rows 0..11)]
CF_W = FD + RC + RC + D
# packed bf16 const layout: [Xt | Mt | PDt | OH 144 | I 128]
CB_W = 3 * FD + RC * RC + L
# per-r param slice: [wt0|wt1|wt2|wt3|wv] 5*64 + bt4 expanded to [L, FD]
WS_W = 5 * D + FD

_nbf16 = ml_dtypes.bfloat16

KB = 3   # k buffers (ACT -> DVE)
WB = 5   # ws slots (DMA -> DVE/PE)
LB = 5   # wl buffers (DVE -> PE)
LAB = 4  # lat buffers (ACT -> DVE)
PRB = 3  # product buffer sets (DVE -> PE)
ZB = 3   # psum z triple-buffer (uses 6 of 8 banks; ps0/ps1 take the rest)
WLAG = 3  # wl(r-WLAG) emitted in DVE iter r
ALAG = 2  # lat(r-ALAG) emitted in ACT iter r


def _vc_m3(r):
    if r < WLAG:
        return 5 * r + 6
    return 6 * r + 4


def _vc_wl(rr):
    if rr <= RC - 1 - WLAG:
        return 6 * (rr + WLAG) - 1
    return 73 - (RC - 1 - rr)


def _vc_g(r):
    if r < WLAG:
        return 5 * r + 2
    return 6 * r


def _build_graph():
    nc = bass.Bass()
    cf_e = nc.declare_dram_parameter("cf", [L, CF_W], F32, isOutput=False)
    cb_e = nc.declare_dram_parameter("cb", [L, CB_W], BF16, isOutput=False)
    Wp_e = nc.declare_dram_parameter("Wp", [RC, L, WS_W], BF16, isOutput=False)
    out_e = nc.declare_dram_parameter("out", [RC, FD], F32, isOutput=True)

    from contextlib import ExitStack

    with ExitStack() as ctx:
        e = ctx.enter_context
        cf = e(nc.sbuf_tensor([L, CF_W], F32))
        cb = e(nc.sbuf_tensor([L, CB_W], BF16))
        Xp = e(nc.sbuf_tensor([L, FD], BF16))
        dist = e(nc.sbuf_tensor([L, FD], F32))
        kbuf = e(nc.sbuf_tensor([L, KB * FD], BF16))
        wsbuf = e(nc.sbuf_tensor([L, WB * WS_W], BF16))
        g = e(nc.sbuf_tensor([L, FD], BF16))
        qb = e(nc.sbuf_tensor([L, PRB * FD], BF16))
        m0b = e(nc.sbuf_tensor([L, PRB * FD], BF16))
        m2b = e(nc.sbuf_tensor([L, PRB * FD], BF16))
        m3b = e(nc.sbuf_tensor([L, PRB * FD], BF16))
        latb = e(nc.sbuf_tensor([L, LAB * FD], BF16))
        wlbuf = e(nc.sbuf_tensor([L, LB * FD], BF16))
        ob = e(nc.sbuf_tensor([RC, FD], F32))
        outt = e(nc.sbuf_tensor([RC, FD], F32))
        wsc = e(nc.sbuf_tensor([L, 512], BF16))
        psz = [e(nc.psum_tensor(f"psz{j}", [L, FD], F32)) for j in range(ZB)]
        ps0 = e(nc.psum_tensor([RC, 512], F32))
        ps1 = e(nc.psum_tensor([RC, 512], F32))
        cfsem = e(nc.semaphore("cfsem"))
        cbsem = e(nc.semaphore("cbsem"))
        wsem = [e(nc.semaphore(f"wsem{j}")) for j in range(WB)]
        asem = e(nc.semaphore("asem"))   # ACT k completions (1/r)
        lsem = e(nc.semaphore("lsem"))   # ACT lat completions (1/r)
        zsem = e(nc.semaphore("zsem"))   # PE z-group completions (1/r)
        msem = e(nc.semaphore("msem"))   # PE out-mm completions (1/r)
        vsem = e(nc.semaphore("vsem"))   # DVE op completions
        osem = e(nc.semaphore("osem"))
        gsem = e(nc.semaphore("gsem"))
        block = e(nc.Block())

        Tt = cf[:, 0:FD]
        Abc = cf[:, FD : FD + RC]
        NASbc = cf[:, FD + RC : FD + 2 * RC]
        LBVt = cf[0:RC, FD + 2 * RC : FD + 2 * RC + D]
        Xt = cb[:, 0:FD]
        Mt = cb[:, FD : 2 * FD]
        PDt = cb[:, 2 * FD : 3 * FD]
        OH3 = cb[:, 3 * FD : 3 * FD + RC * RC].rearrange("p (r m) -> p r m", r=RC)
        Ident = cb[:, 3 * FD + RC * RC :]

        def r3(ap):
            return ap.rearrange("p (b d) -> p b d", b=BC)

        def kslot(r):
            return kbuf[:, (r % KB) * FD : (r % KB + 1) * FD]

        def wslot(r):
            return wsbuf[:, (r % WB) * WS_W : (r % WB + 1) * WS_W]

        def wbc(r, e):
            base = (r % WB) * WS_W + e * D
            return wsbuf[:, base : base + D].unsqueeze(1).broadcast_to([L, BC, D])

        def bt4x(r):
            base = (r % WB) * WS_W + 5 * D
            return wsbuf[:, base : base + FD]

        def latslot(r):
            return latb[:, (r % LAB) * FD : (r % LAB + 1) * FD]

        def wlslot(r):
            return wlbuf[:, (r % LB) * FD : (r % LB + 1) * FD]

        def prod(buf, r):
            return buf[:, (r % PRB) * FD : (r % PRB + 1) * FD]

        CBH = CB_W // 2

        @block.sync
        def _(sp):
            for r in range(RC):
                if r == 1:
                    sp.dma_start(
                        out=cb[:, 0:CBH], in_=cb_e[:, 0:CBH]
                    ).then_inc(cbsem, 16)
                    sp.dma_start(
                        out=cb[:, CBH:], in_=cb_e[:, CBH:]
                    ).then_inc(cbsem, 16)
                if r >= WB:
                    # ws slot readers: DVE wl(r-WB) is the last DVE read;
                    # PE z-group(r-WB) reads bt4x
                    sp.wait_ge(vsem, _vc_wl(r - WB))
                    sp.wait_ge(zsem, r - WB + 1)
                sp.dma_start(out=wslot(r), in_=Wp_e[r, :, :]).then_inc(
                    wsem[r % WB], 16
                )
            sp.wait_ge(vsem, 76)
            sp.dma_start(out=out_e[:, :], in_=outt[:, :]).then_inc(osem, 16)

        @block.scalar
        def _(act):
            act.dma_start(out=cf[:, :], in_=cf_e[:, :]).then_inc(cfsem, 16)
            act.wait_ge(cfsem, 16)
            for r in range(RC):
                nc.scalar.activation(
                    dist[:, :], Tt, AF.Abs,
                    bias=NASbc[:, r : r + 1], scale=Abc[:, r : r + 1],
                )
                if r >= KB:
                    act.wait_ge(vsem, _vc_g(r - KB))
                nc.scalar.activation(
                    kslot(r), dist[:, :], AF.Exp, scale=-1.0
                ).then_inc(asem, 1)
                if r >= ALAG:
                    rr = r - ALAG
                    act.wait_ge(zsem, rr + 1)
                    if rr >= LAB:
                        act.wait_ge(vsem, _vc_wl(rr - LAB))
                    nc.scalar.activation(
                        latslot(rr), psz[rr % ZB][:, :], AF.Relu
                    ).then_inc(lsem, 1)
            for rr in range(RC - ALAG, RC):
                act.wait_ge(zsem, rr + 1)
                nc.scalar.activation(
                    latslot(rr), psz[rr % ZB][:, :], AF.Relu
                ).then_inc(lsem, 1)

        @block.vector
        def _(ve):
            ve.wait_ge(cbsem, 32)
            nc.vector.tensor_scalar_max(Xp[:, :], Xt, 0.0).then_inc(vsem, 1)
            for r in range(RC):
                if r >= WLAG:
                    rr = r - WLAG
                    # wl(rr): lat(rr) is WLAG iterations old; the lsem wait
                    # also implies zsem >= r-2, covering product-slot reuse
                    ve.wait_ge(lsem, rr + 1)
                    if rr >= LB:
                        ve.wait_ge(msem, rr - LB + 1)
                    nc.vector.tensor_tensor(
                        r3(wlslot(rr)), r3(latslot(rr)), wbc(rr, 4),
                        OP.mult,
                    ).then_inc(vsem, 1)
                ve.wait_ge(asem, r + 1)
                nc.vector.tensor_mul(g[:, :], Xp[:, :], kslot(r)).then_inc(vsem, 1)
                ve.wait_ge(wsem[r % WB], 16 * (r // WB + 1))
                nc.vector.tensor_tensor(
                    r3(prod(qb, r)), r3(g[:, :]), wbc(r, 1), OP.mult
                ).then_inc(vsem, 1)
                nc.vector.tensor_tensor(
                    r3(prod(m0b, r)), r3(Xt), wbc(r, 0), OP.mult
                ).then_inc(vsem, 1)
                nc.vector.tensor_tensor(
                    r3(prod(m2b, r)), r3(Mt), wbc(r, 2), OP.mult
                ).then_inc(vsem, 1)
                nc.vector.tensor_tensor(
                    r3(prod(m3b, r)), r3(PDt), wbc(r, 3), OP.mult
                ).then_inc(vsem, 1)
            # final wl's + tail
            for rr in range(RC - WLAG, RC):
                ve.wait_ge(lsem, rr + 1)
                ve.wait_ge(msem, rr - LB + 1)
                nc.vector.tensor_tensor(
                    r3(wlslot(rr)), r3(latslot(rr)), wbc(rr, 4), OP.mult
                ).then_inc(vsem, 1)
            ve.wait_ge(msem, RC)
            ve.wait_ge(cfsem, 16)
            lb3 = LBVt.unsqueeze(1).broadcast_to([RC, BC // 2, D])
            for h, ps in enumerate((ps0, ps1)):
                ob3 = r3(ob[:, :])[:, h * (BC // 2) : (h + 1) * (BC // 2), :]
                ps3 = ps[:, :].rearrange("p (b d) -> p b d", b=BC // 2)
                nc.vector.scalar_tensor_tensor(
                    ob3, ps3, 0.0, lb3, OP.add, OP.add
                ).then_inc(vsem, 1)
            nc.vector.tensor_scalar_max(outt[:, :], ob[:, :], 0.0).then_inc(vsem, 1)

        @block.gpsimd
        def _(gp):
            nc.gpsimd.memset(wsc[:, :], 1.0).then_inc(gsem, 1)

        @block.tensor
        def _(te):
            # HAM warmup phase 1: matmuls on a memset scratch starting right
            # after the preamble, keeping the PE busy through the ~12us DMA
            # wait so it never sits in a >3.4us idle window. Results are
            # never read; ps0 is reset by the real start=True.
            te.wait_ge(gsem, 1)
            for _w in range(26):
                nc.tensor.matmul(
                    ps0[:, :], wsc[:, 0:RC], wsc[:, :],
                    start=True, stop=True, skip_group_check=True,
                )
            te.wait_ge(cbsem, 32)
            # phase 2: bridge from DMA arrival to the first real z-group
            for _w in range(12):
                nc.tensor.matmul(
                    ps0[:, :], OH3[:, 0, :], cb[:, 0:512],
                    start=True, stop=True, skip_group_check=True,
                )
            for r in range(RC):
                te.wait_ge(vsem, _vc_m3(r))
                if r >= ZB:
                    te.wait_ge(lsem, r - ZB + 1)
                pz = psz[r % ZB]
                # alternate PSUM banks between consecutive matmuls so the
                # drain of one overlaps the fill of the next
                for pb, first, last in (
                    (None, True, False),
                    (qb, False, False),
                    (m0b, False, False),
                    (m2b, False, False),
                    (m3b, False, True),
                ):
                    for h in range(2):
                        c0, c1 = h * 512, (h + 1) * 512
                        rhs = (
                            bt4x(r)[:, c0:c1]
                            if pb is None
                            else prod(pb, r)[:, c0:c1]
                        )
                        mm = nc.tensor.matmul(
                            pz[:, c0:c1], Ident, rhs,
                            start=first, stop=last, skip_group_check=True,
                        )
                        if last and h == 1:
                            mm.then_inc(zsem, 1)
                if r >= WLAG:
                    rr = r - WLAG
                    te.wait_ge(vsem, _vc_wl(rr))
                    wl = wlslot(rr)
                    nc.tensor.matmul(
                        ps0[:, :], OH3[:, rr, :], wl[:, 0:512],
                        start=(rr == 0), stop=False,
                        skip_group_check=True,
                    )
                    nc.tensor.matmul(
                        ps1[:, :], OH3[:, rr, :], wl[:, 512:1024],
                        start=(rr == 0), stop=False,
                        skip_group_check=True,
                    ).then_inc(msem, 1)
            for rr in range(RC - WLAG, RC):
                te.wait_ge(vsem, _vc_wl(rr))
                wl = wlslot(rr)
                nc.tensor.matmul(
                    ps0[:, :], OH3[:, rr, :], wl[:, 0:512],
                    start=False, stop=(rr == RC - 1), skip_group_check=True,
                )
                nc.tensor.matmul(
                    ps1[:, :], OH3[:, rr, :], wl[:, 512:1024],
                    start=False, stop=(rr == RC - 1), skip_group_check=True,
                ).then_inc(msem, 1)

    return nc


_CACHE = {}


def kernel(X, T, M, PD, alpha, w_v, w_t, b_t, b_v, ref_time):
    X = np.asarray(X, np.float32)
    T = np.asarray(T, np.float32)
    M = np.asarray(M, np.float32)
    PD = np.asarray(PD, np.float32)
    alpha = np.asarray(alpha, np.float32)
    w_v = np.asarray(w_v, np.float32)
    w_t = np.asarray(w_t, np.float32)
    b_t = np.asarray(b_t, np.float32)
    b_v = np.asarray(b_v, np.float32)
    ref_time = np.asarray(ref_time, np.float32)

    a = np.maximum(alpha.reshape(R), 0.0)
    s_ref = ref_time.reshape(R)
    nas = -(a * s_ref)
    bt4 = 4.0 * b_t[..., 0]              # [R, L, D]
    lbv = float(L) * b_v[:, 0, :]        # [R, D]

    # per-r params: [wt0|wt1|wt2|wt3|wv] (5*D) + bt4 expanded to [L, FD]
    wts = np.stack(
        [w_t[..., 0], w_t[..., 1], w_t[..., 2], w_t[..., 3], w_v], axis=2
    )                                     # [R, L, 5, D]
    bt4x = np.broadcast_to(bt4[:, :, None, :], (R, L, BC, D)).reshape(R, L, FD)
    wpack = np.concatenate(
        [wts.reshape(R, L, 5 * D), bt4x], axis=2
    )                                     # [R, L, WS_W]

    oh = np.zeros((L, RC, RC), np.float32)
    for r in range(RC):
        oh[:, r, r] = 1.0
    ident = np.eye(L, dtype=np.float32)

    if "nc" not in _CACHE:
        _CACHE["nc"] = _build_graph()
    nc = _CACHE["nc"]

    in_maps = []
    for c in range(8):
        b0 = (c // NR) * BC
        r0 = (c % NR) * RC
        tr = lambda x: np.ascontiguousarray(
            x[b0 : b0 + BC].transpose(1, 0, 2).reshape(L, FD)
        )
        cf = np.zeros((L, CF_W), np.float32)
        cf[:, 0:FD] = tr(T)
        cf[:, FD : FD + RC] = a[r0 : r0 + RC]
        cf[:, FD + RC : FD + 2 * RC] = nas[r0 : r0 + RC]
        cf[0:RC, FD + 2 * RC : FD + 2 * RC + D] = lbv[r0 : r0 + RC]
        cbf = np.zeros((L, CB_W), np.float32)
        cbf[:, 0:FD] = tr(X)
        cbf[:, FD : 2 * FD] = tr(M)
        cbf[:, 2 * FD : 3 * FD] = tr(PD)
        cbf[:, 3 * FD : 3 * FD + RC * RC] = oh.reshape(L, RC * RC)
        cbf[:, 3 * FD + RC * RC :] = ident
        in_maps.append(
            {
                "cf": cf,
                "cb": cbf.astype(_nbf16),
                "Wp": np.ascontiguousarray(wpack[r0 : r0 + RC]).astype(_nbf16),
            }
        )

    trace = bool(os.environ.get("BASS_KERNEL_TRACE"))
    kw = {}
    if trace:
        tmpdir = os.environ.get("BASS_KERNEL_TRACE_DIR") or None
        kw = dict(trace=True, tmpdir=tmpdir)
    res = run_bass_kernel_spmd(nc, in_maps, core_ids=list(range(8)), **kw)
    if trace:
        _CACHE["exec_time_ns"] = res.exec_time_ns
        print(f"HW exec time: {res.exec_time_ns} ns")

    out = np.zeros((B, R, D), np.float32)
    for c in range(8):
        b0 = (c // NR) * BC
        r0 = (c % NR) * RC
        o = np.asarray(res.results[c]["out"], np.float32).reshape(RC, BC, D)
        out[b0 : b0 + BC, r0 : r0 + RC] = o.transpose(1, 0, 2)
    return out



# revision 7
# speedup vs baseline: 1.0691x; 1.0691x over previous
"""ALNN layer kernel for 8 TRN2 NeuronCores (raw Bass, explicit semaphores).

out[b,r,d] = relu( sum_l w_v[r,l,d]*relu(z[b,r,l,d]) + L*b_v[r,d] )
z = wt0*X + wt1*relu(X)*k + wt2*M + wt3*PD + 4*bt
k = exp(-relu(alpha_r)*|T - s_r|)        (uses relu(X*k) == relu(X)*k, k>0)

Sharding: B split 2 ways x R dealt into 4 buckets -> 8 cores, 16 b x 12 r
each. Cores c and c+4 share bucket c%4. Within a bucket the r's with
relu(alpha)=0 come first; for those, k == 1 so ABS/EXP/g are skipped
(q = relu(X)*wt1 directly). The graph is SPMD-uniform: the fast-path
iteration count is min over buckets of their zero count, so every core
runs the same instruction stream (extra zeros just take the slow path,
which is still correct since exp(-0*dist) == 1).

Per-core layout: partitions = L(128), free = (b=16, d=64) = 1024.

v3 engine plan (v2 profiling: all three compute engines ~balanced at
~3.3us/iter but kernel 68.5us vs DVE busy 45.8us -> the win is pipeline
fill + DMA: bt4 was host-expanded 16x to [L,1024] per r, 3.1MB/core of
the 5.6MB DMA, pacing the whole kernel):
 - DVE: the 4-5 weighted products per iter + wl (bf16 2x mode), plus the
   final PSUM->ob adds. ~41us busy = the roofline for this kernel shape.
 - PE:  z accumulated in PSUM: bt4 reconstructed from a 16KB transposed
   copy via a one-hot-d matmul (kills the bt4x DMA), then q/m0/m2/m3 via
   identity matmuls; final L-reduction as one-hot-column matmuls.
 - ACT: dist=abs, k=exp (slow iters only), lat=relu(PSUM z), final relu.
 - All inputs DMA'd up front (2.7MB/core), ordered so X/W arrive first;
   DVE starts ~3us in.

Raw bass: this toolchain's walrus allows at most ONE attached sync-wait
per compute instruction, so cross-engine deps use standalone wait_ge
instructions; DMA completion uses dedicated semaphores per stream.
"""

import os
import numpy as np
import ml_dtypes

import concourse.bass as bass
import concourse.mybir as mybir
from concourse.bass_utils import run_bass_kernel_spmd

AF = mybir.ActivationFunctionType
OP = mybir.AluOpType
BF16 = mybir.dt.bfloat16
F32 = mybir.dt.float32

B, R, L, D = 32, 48, 128, 64
NB, NK = 2, 4              # b-halves x r-buckets = 8 cores
BC, RC = B // NB, R // NK  # 16 b, 12 r per core
FD = BC * D                # 1024 free elems

# packed f32 const layout: [Tt 1024 | Abc 12 | NASbc 12 | LBV 64(rows 0..11)]
CF_W = FD + RC + RC + D
# packed bf16 const layout: [Xt | Mt | PDt | OH 144 | I 128]
CB_W = 3 * FD + RC * RC + L
WS_W = 5 * D               # per-iter param slice: [wt0|wt1|wt2|wt3|wv]

_nbf16 = ml_dtypes.bfloat16

LB = 5   # wl buffers (DVE -> PE)
LAB = 4  # lat buffers (ACT -> DVE)
PRB = 3  # product buffer sets (DVE -> PE)
ZB = 3   # psum z triple-buffer (6 of 8 banks; ps0/ps1 take the rest)
WLAG = 3  # wl(i-WLAG) emitted in DVE iter i
ALAG = 2  # lat(i-ALAG) emitted in ACT iter i
NWARM = 16  # PE warmup matmuls (keep PE out of low p-state until work)


def _dve_schedule(nfast):
    """DVE emission order; must match the @block.vector body exactly."""
    sched = [("xp", 0)]
    for i in range(RC):
        if i >= WLAG:
            sched.append(("wl", i - WLAG))
        if i >= nfast:
            sched.append(("g", i))
        sched.append(("q", i))
        sched.append(("m0", i))
        sched.append(("m2", i))
        sched.append(("m3", i))
    for rr in range(RC - WLAG, RC):
        sched.append(("wl", rr))
    sched.append(("stt", 0))
    sched.append(("stt", 1))
    return sched


def _build_graph(nfast, detect_races=True):
    nslow = RC - nfast
    ksl = max(nslow, 1)
    sched = _dve_schedule(nfast)
    C = {key: idx + 1 for idx, key in enumerate(sched)}

    nc = bass.Bass(detect_race_conditions=detect_races)
    cf_e = nc.declare_dram_parameter("cf", [L, CF_W], F32, isOutput=False)
    cb_e = nc.declare_dram_parameter("cb", [L, CB_W], BF16, isOutput=False)
    W_e = nc.declare_dram_parameter("W", [L, RC * WS_W], BF16, isOutput=False)
    bt_e = nc.declare_dram_parameter("BT", [L, RC * L], BF16, isOutput=False)
    oh_e = nc.declare_dram_parameter("OHD", [L, 512], BF16, isOutput=False)
    out_e = nc.declare_dram_parameter("out", [RC, FD], F32, isOutput=True)

    from contextlib import ExitStack

    with ExitStack() as ctx:
        e = ctx.enter_context
        cf = e(nc.sbuf_tensor([L, CF_W], F32))
        cb = e(nc.sbuf_tensor([L, CB_W], BF16))
        Wb = e(nc.sbuf_tensor([L, RC * WS_W], BF16))
        BT = e(nc.sbuf_tensor([L, RC * L], BF16))
        OHD = e(nc.sbuf_tensor([L, 512], BF16))
        Xp = e(nc.sbuf_tensor([L, FD], BF16))
        dist = e(nc.sbuf_tensor([L, FD], F32))
        kbuf = e(nc.sbuf_tensor([L, ksl * FD], BF16))
        g = e(nc.sbuf_tensor([L, FD], BF16))
        qb = e(nc.sbuf_tensor([L, PRB * FD], BF16))
        m0b = e(nc.sbuf_tensor([L, PRB * FD], BF16))
        m2b = e(nc.sbuf_tensor([L, PRB * FD], BF16))
        m3b = e(nc.sbuf_tensor([L, PRB * FD], BF16))
        latb = e(nc.sbuf_tensor([L, LAB * FD], BF16))
        wlbuf = e(nc.sbuf_tensor([L, LB * FD], BF16))
        ob = e(nc.sbuf_tensor([RC, FD], F32))
        outt = e(nc.sbuf_tensor([RC, FD], F32))
        wsc = e(nc.sbuf_tensor([L, 512], BF16))
        psz = [e(nc.psum_tensor(f"psz{j}", [L, FD], F32)) for j in range(ZB)]
        ps0 = e(nc.psum_tensor([RC, 512], F32))
        ps1 = e(nc.psum_tensor([RC, 512], F32))
        cfsem = e(nc.semaphore("cfsem"))
        cbsem = e(nc.semaphore("cbsem"))    # cb X part
        cbmsem = e(nc.semaphore("cbmsem"))  # cb M+PD part
        cbtsem = e(nc.semaphore("cbtsem"))  # cb OH3+Ident part
        wsem0 = e(nc.semaphore("wsem0"))    # W iter 0
        wsema = e(nc.semaphore("wsema"))    # W iters 1-5
        wsemb = e(nc.semaphore("wsemb"))    # W iters 6-11
        ohsem = e(nc.semaphore("ohsem"))    # OHD
        btsem = e(nc.semaphore("btsem"))    # BT
        asem = e(nc.semaphore("asem"))      # ACT k completions
        lsem = e(nc.semaphore("lsem"))      # ACT lat/final relu completions
        zsem = e(nc.semaphore("zsem"))      # PE z-group completions (1/iter)
        msem = e(nc.semaphore("msem"))      # PE out-mm completions (1/iter)
        vsem = e(nc.semaphore("vsem"))      # DVE op completions
        osem = e(nc.semaphore("osem"))
        gsem = e(nc.semaphore("gsem"))
        block = e(nc.Block())

        Tt = cf[:, 0:FD]
        Abc = cf[:, FD : FD + RC]
        NASbc = cf[:, FD + RC : FD + 2 * RC]
        LBVt = cf[0:RC, FD + 2 * RC : FD + 2 * RC + D]
        Xt = cb[:, 0:FD]
        Mt = cb[:, FD : 2 * FD]
        PDt = cb[:, 2 * FD : 3 * FD]
        OH3 = cb[:, 3 * FD : 3 * FD + RC * RC].rearrange("p (r m) -> p r m", r=RC)
        Ident = cb[:, 3 * FD + RC * RC :]

        def r3(ap):
            return ap.rearrange("p (b d) -> p b d", b=BC)

        def kslot(j):
            return kbuf[:, (j % ksl) * FD : (j % ksl + 1) * FD]

        def wbc(i, ei):
            base = i * WS_W + ei * D
            return Wb[:, base : base + D].unsqueeze(1).broadcast_to([L, BC, D])

        def btsl(i):
            return BT[0:64, i * L : (i + 1) * L]

        def latslot(rr):
            return latb[:, (rr % LAB) * FD : (rr % LAB + 1) * FD]

        def wlslot(rr):
            return wlbuf[:, (rr % LB) * FD : (rr % LB + 1) * FD]

        def prod(buf, i):
            return buf[:, (i % PRB) * FD : (i % PRB + 1) * FD]

        @block.sync
        def _(sp):
            sp.dma_start(out=cb[:, 0:FD], in_=cb_e[:, 0:FD]).then_inc(cbsem, 16)
            sp.dma_start(out=Wb[:, 0:WS_W], in_=W_e[:, 0:WS_W]).then_inc(wsem0, 16)
            sp.dma_start(out=cb[:, FD : 3 * FD], in_=cb_e[:, FD : 3 * FD]).then_inc(
                cbmsem, 16
            )
            sp.dma_start(
                out=Wb[:, WS_W : 6 * WS_W], in_=W_e[:, WS_W : 6 * WS_W]
            ).then_inc(wsema, 16)
            sp.dma_start(out=cb[:, 3 * FD :], in_=cb_e[:, 3 * FD :]).then_inc(
                cbtsem, 16
            )
            sp.dma_start(out=OHD[:, :], in_=oh_e[:, :]).then_inc(ohsem, 16)
            sp.dma_start(out=BT[:, :], in_=bt_e[:, :]).then_inc(btsem, 16)
            sp.dma_start(
                out=Wb[:, 6 * WS_W :], in_=W_e[:, 6 * WS_W :]
            ).then_inc(wsemb, 16)
            sp.wait_ge(lsem, RC + 1)
            sp.dma_start(out=out_e[:, :], in_=outt[:, :]).then_inc(osem, 16)

        @block.scalar
        def _(act):
            act.dma_start(out=cf[:, :], in_=cf_e[:, :]).then_inc(cfsem, 16)
            act.wait_ge(cfsem, 16)
            for i in range(RC):
                if i < nslow:
                    si = nfast + i
                    nc.scalar.activation(
                        dist[:, :], Tt, AF.Abs,
                        bias=NASbc[:, si : si + 1], scale=Abc[:, si : si + 1],
                    )
                    nc.scalar.activation(
                        kslot(i), dist[:, :], AF.Exp, scale=-1.0
                    ).then_inc(asem, 1)
                if i >= ALAG:
                    rr = i - ALAG
                    act.wait_ge(zsem, rr + 1)
                    if rr >= LAB:
                        act.wait_ge(vsem, C[("wl", rr - LAB)])
                    nc.scalar.activation(
                        latslot(rr), psz[rr % ZB][:, :], AF.Relu
                    ).then_inc(lsem, 1)
            for rr in range(RC - ALAG, RC):
                act.wait_ge(zsem, rr + 1)
                act.wait_ge(vsem, C[("wl", rr - LAB)])
                nc.scalar.activation(
                    latslot(rr), psz[rr % ZB][:, :], AF.Relu
                ).then_inc(lsem, 1)
            act.wait_ge(vsem, C[("stt", 1)])
            nc.scalar.activation(outt[:, :], ob[:, :], AF.Relu).then_inc(lsem, 1)

        @block.vector
        def _(ve):
            ve.wait_ge(cbsem, 16)
            nc.vector.tensor_scalar_max(Xp[:, :], Xt, 0.0).then_inc(vsem, 1)
            for i in range(RC):
                if i >= WLAG:
                    rr = i - WLAG
                    ve.wait_ge(lsem, rr + 1)
                    if rr >= LB:
                        ve.wait_ge(msem, rr - LB + 1)
                    nc.vector.tensor_tensor(
                        r3(wlslot(rr)), r3(latslot(rr)), wbc(rr, 4), OP.mult
                    ).then_inc(vsem, 1)
                if i >= nfast:
                    ve.wait_ge(asem, i - nfast + 1)
                    nc.vector.tensor_mul(g[:, :], Xp[:, :], kslot(i - nfast)).then_inc(
                        vsem, 1
                    )
                    qsrc = g[:, :]
                else:
                    qsrc = Xp[:, :]
                if i == 0:
                    ve.wait_ge(wsem0, 16)
                elif i == 1:
                    ve.wait_ge(wsema, 16)
                elif i == 6:
                    ve.wait_ge(wsemb, 16)
                nc.vector.tensor_tensor(
                    r3(prod(qb, i)), r3(qsrc), wbc(i, 1), OP.mult
                ).then_inc(vsem, 1)
                nc.vector.tensor_tensor(
                    r3(prod(m0b, i)), r3(Xt), wbc(i, 0), OP.mult
                ).then_inc(vsem, 1)
                if i == 0:
                    ve.wait_ge(cbmsem, 16)
                nc.vector.tensor_tensor(
                    r3(prod(m2b, i)), r3(Mt), wbc(i, 2), OP.mult
                ).then_inc(vsem, 1)
                nc.vector.tensor_tensor(
                    r3(prod(m3b, i)), r3(PDt), wbc(i, 3), OP.mult
                ).then_inc(vsem, 1)
            for rr in range(RC - WLAG, RC):
                ve.wait_ge(lsem, rr + 1)
                ve.wait_ge(msem, rr - LB + 1)
                nc.vector.tensor_tensor(
                    r3(wlslot(rr)), r3(latslot(rr)), wbc(rr, 4), OP.mult
                ).then_inc(vsem, 1)
            ve.wait_ge(msem, RC)
            lb3 = LBVt.unsqueeze(1).broadcast_to([RC, BC // 2, D])
            for h, ps in enumerate((ps0, ps1)):
                ob3 = r3(ob[:, :])[:, h * (BC // 2) : (h + 1) * (BC // 2), :]
                ps3 = ps[:, :].rearrange("p (b d) -> p b d", b=BC // 2)
                nc.vector.scalar_tensor_tensor(
                    ob3, ps3, 0.0, lb3, OP.add, OP.add
                ).then_inc(vsem, 1)

        @block.gpsimd
        def _(gp):
            nc.gpsimd.memset(wsc[:, :], 1.0).then_inc(gsem, 1)

        @block.tensor
        def _(te):
            # warmup: keep the PE out of its low p-state until real work
            # arrives (~4.5us in). Results never read; ps0 reset by the
            # real start=True.
            te.wait_ge(gsem, 1)
            for _w in range(NWARM):
                nc.tensor.matmul(
                    ps0[:, :], wsc[:, 0:RC], wsc[:, :],
                    start=True, stop=True, skip_group_check=True,
                )
            te.wait_ge(ohsem, 16)
            te.wait_ge(btsem, 16)
            te.wait_ge(cbtsem, 16)
            for i in range(RC):
                te.wait_ge(vsem, C[("m3", i)])
                if i >= ZB:
                    te.wait_ge(lsem, i - ZB + 1)
                pz = psz[i % ZB]
                for h in range(2):
                    nc.tensor.matmul(
                        pz[:, h * 512 : (h + 1) * 512], btsl(i), OHD[0:64, :],
                        start=True, stop=False, skip_group_check=True,
                    )
                for pb, last in ((qb, False), (m0b, False), (m2b, False), (m3b, True)):
                    for h in range(2):
                        c0, c1 = h * 512, (h + 1) * 512
                        mm = nc.tensor.matmul(
                            pz[:, c0:c1], Ident, prod(pb, i)[:, c0:c1],
                            start=False, stop=last, skip_group_check=True,
                        )
                        if last and h == 1:
                            mm.then_inc(zsem, 1)
                if i >= WLAG:
                    rr = i - WLAG
                    te.wait_ge(vsem, C[("wl", rr)])
                    wl = wlslot(rr)
                    nc.tensor.matmul(
                        ps0[:, :], OH3[:, rr, :], wl[:, 0:512],
                        start=(rr == 0), stop=False, skip_group_check=True,
                    )
                    nc.tensor.matmul(
                        ps1[:, :], OH3[:, rr, :], wl[:, 512:1024],
                        start=(rr == 0), stop=False, skip_group_check=True,
                    ).then_inc(msem, 1)
            for rr in range(RC - WLAG, RC):
                te.wait_ge(vsem, C[("wl", rr)])
                wl = wlslot(rr)
                nc.tensor.matmul(
                    ps0[:, :], OH3[:, rr, :], wl[:, 0:512],
                    start=False, stop=(rr == RC - 1), skip_group_check=True,
                )
                nc.tensor.matmul(
                    ps1[:, :], OH3[:, rr, :], wl[:, 512:1024],
                    start=False, stop=(rr == RC - 1), skip_group_check=True,
                ).then_inc(msem, 1)

    return nc


_CACHE = {}


def _buckets(a):
    """Deal r-indices into NK buckets of RC, zeros-first in each bucket.
    Returns (buckets, nfast): nfast = min zero-count across buckets."""
    zeros = [r for r in range(R) if a[r] == 0.0]
    pos = [r for r in range(R) if a[r] != 0.0]
    buckets = [[] for _ in range(NK)]
    for j, r in enumerate(zeros):
        buckets[j % NK].append(r)
    zc = [len(b) for b in buckets]
    pi = 0
    for k in range(NK):
        while len(buckets[k]) < RC:
            buckets[k].append(pos[pi])
            pi += 1
    nfast = min(min(zc), RC)
    return buckets, nfast


def _prepare(X, T, M, PD, alpha, w_v, w_t, b_t, b_v, ref_time):
    """Pack full inputs into per-core DRAM parameter maps.
    Returns (nfast, buckets, in_maps)."""
    a = np.maximum(alpha.reshape(R), 0.0)
    s_ref = ref_time.reshape(R)
    nas = -(a * s_ref)
    bt4 = 4.0 * b_t[..., 0]              # [R, L, D]
    lbv = float(L) * b_v[:, 0, :]        # [R, D]

    buckets, nfast = _buckets(a)

    # per-r params: [wt0|wt1|wt2|wt3|wv] (5*D per iter)
    wts = np.stack(
        [w_t[..., 0], w_t[..., 1], w_t[..., 2], w_t[..., 3], w_v], axis=2
    )                                     # [R, L, 5, D]

    oh = np.zeros((L, RC, RC), np.float32)
    for r in range(RC):
        oh[:, r, r] = 1.0
    ident = np.eye(L, dtype=np.float32)
    ohd = np.zeros((L, 512), np.float32)
    for b in range(8):
        for d in range(64):
            ohd[d, b * 64 + d] = 1.0

    in_maps = []
    for c in range(8):
        b0 = (c // NK) * BC
        rl = buckets[c % NK]
        tr = lambda x: np.ascontiguousarray(
            x[b0 : b0 + BC].transpose(1, 0, 2).reshape(L, FD)
        )
        cf = np.zeros((L, CF_W), np.float32)
        cf[:, 0:FD] = tr(T)
        cf[:, FD : FD + RC] = a[rl]
        cf[:, FD + RC : FD + 2 * RC] = nas[rl]
        cf[0:RC, FD + 2 * RC : FD + 2 * RC + D] = lbv[rl]
        cbf = np.zeros((L, CB_W), np.float32)
        cbf[:, 0:FD] = tr(X)
        cbf[:, FD : 2 * FD] = tr(M)
        cbf[:, 2 * FD : 3 * FD] = tr(PD)
        cbf[:, 3 * FD : 3 * FD + RC * RC] = oh.reshape(L, RC * RC)
        cbf[:, 3 * FD + RC * RC :] = ident
        wp = wts[rl].reshape(RC, L, 5 * D).transpose(1, 0, 2).reshape(L, RC * WS_W)
        btp = np.zeros((L, RC * L), np.float32)
        for i, r in enumerate(rl):
            btp[0:D, i * L : (i + 1) * L] = bt4[r].T
        in_maps.append(
            {
                "cf": cf,
                "cb": cbf.astype(_nbf16),
                "W": np.ascontiguousarray(wp).astype(_nbf16),
                "BT": btp.astype(_nbf16),
                "OHD": ohd.astype(_nbf16),
            }
        )
    return nfast, buckets, in_maps


def kernel(X, T, M, PD, alpha, w_v, w_t, b_t, b_v, ref_time):
    X = np.asarray(X, np.float32)
    T = np.asarray(T, np.float32)
    M = np.asarray(M, np.float32)
    PD = np.asarray(PD, np.float32)
    alpha = np.asarray(alpha, np.float32)
    w_v = np.asarray(w_v, np.float32)
    w_t = np.asarray(w_t, np.float32)
    b_t = np.asarray(b_t, np.float32)
    b_v = np.asarray(b_v, np.float32)
    ref_time = np.asarray(ref_time, np.float32)

    nfast, buckets, in_maps = _prepare(
        X, T, M, PD, alpha, w_v, w_t, b_t, b_v, ref_time
    )

    if nfast not in _CACHE:
        _CACHE[nfast] = _build_graph(nfast)
    nc = _CACHE[nfast]

    trace = bool(os.environ.get("BASS_KERNEL_TRACE"))
    kw = {}
    if trace:
        tmpdir = os.environ.get("BASS_KERNEL_TRACE_DIR") or None
        kw = dict(trace=True, tmpdir=tmpdir)
    res = run_bass_kernel_spmd(nc, in_maps, core_ids=list(range(8)), **kw)
    if trace:
        _CACHE["exec_time_ns"] = res.exec_time_ns
        print(f"HW exec time: {res.exec_time_ns} ns")

    out = np.zeros((B, R, D), np.float32)
    for c in range(8):
        b0 = (c // NK) * BC
        rl = buckets[c % NK]
        o = np.asarray(res.results[c]["out"], np.float32).reshape(RC, BC, D)
        for i, r in enumerate(rl):
            out[b0 : b0 + BC, r] = o[i]
    return out
